# revision 29
# baseline (speedup 1.0000x reference)
"""CorrNoise kernel for 8x TRN2 NeuronCores.

Reference computation: center/normalize ref over batch -> per-dim (l x l)
correlation -> eigh -> out[d] = (Q*sqrt(max(eig,0)))[d] @ noise[d].

Split of work:
  * corr + eigh run on HOST with jax on CPU, mirroring the reference ops
    bit-exactly.  This is forced: (a) eigh has no neuron lowering at all;
    (b) LAPACK eigenvector SIGNS are implementation-defined and flip under
    ~1e-7 input perturbations, and the output is sign-sensitive, so the
    eigh input must be bit-identical to the reference's and the eigh must
    be the same LAPACK build (jnp.linalg.eigh on CPU).
  * The post-eigh work - 512 independent (128x128)@(128x256) GEMMs - runs
    on the 8 NeuronCores, sharded by dim (64 per core).

Device kernel design (int8 wire format; measured ~34us vs 63.5us
fp16-hi/lo baseline):
  * DMA is the binding resource, so precision is traded for bytes
    against the rel_err<2e-2 gate: QS ships fp16, noise ships int8
    (scale 32, clip +-127), output returns int8 (scale 32).  6.3
    MB/core HBM traffic (vs 21 baseline).  The PE consumes noise
    upconverted to fp16 (int8 values are exact in fp16; the 1/32 noise
    and 32x output scales cancel, so no on-chip scaling at all).
    fp32 -> int8 conversions saturate+round-to-nearest on HW
    (verified), matching the numpy model.  End-to-end rel err 1.327e-2
    (inputs are deterministic seed-0, so the harness sees this value).
  * Noise upconversion: compute engines are too slow to convert all of
    it (DVE ~230G elem/s clean, less under contention; GpSimd 35G), so
    most groups use SWDGE cast-load DMAs (int8 HBM -> fp16 SBUF).  A
    cast-DMA costs the fp16-side bytes on the shared SDMA pool
    (measured), so one group (OFF) ships plain int8 and DVE upconverts
    it, trimming the pool (~18us saturated stream).  Offloading a
    second group measured slower.
  * 4 matmuls share a 2-bank [128,1024] fp32 PSUM tile; one saturating
    PSUM->int8 copy per quad, alternating ACT / DVE.
  * Ring balance: qs loads alternate the two HWDGE rings (SP/ACT),
    noise cast-loads ride SWDGE, stores ride SP.  The last group's
    noise loads in 4 chunks and its output streams out per quad, so
    the pipeline tail after the stream ends is short.  8 groups of 8
    dims measured faster than 4x16 (earlier stores) and 16x4 (too many
    DMA issues).
  * All input/output tiles are SBUF-resident: no backward scheduling
    edges, input DMAs never wait on compute.
"""

import numpy as np

EPS = 1e-5
SIZE = 128   # l: corr matrices are SIZE x SIZE
DIM = 512    # d: number of independent feature dims
BATCH = 256  # b
NCORES = 8
DPC = DIM // NCORES  # dims per core
GRP = 8              # dims per load/store group
NGRP = DPC // GRP
QSCALE = 32.0        # int8 quantization scale for noise and output

_cache = {}


def _host_qs(ref: np.ndarray) -> np.ndarray:
    """Bit-exact mirror of the reference's pre-matmul stages on jax CPU.

    Returns QS = Ds[:, None, :] * Qs with shape (DIM, SIZE, SIZE), fp32.
    """
    import jax
    import jax.numpy as jnp

    cpu = jax.devices("cpu")[0]
    with jax.default_device(cpu):
        refj = jnp.asarray(np.asarray(ref, dtype=np.float32))
        x = refj - refj.mean(axis=0, keepdims=True)
        x = x / (jnp.linalg.norm(x, axis=0, keepdims=True) + EPS)
        x = jnp.transpose(x, (2, 1, 0))  # (d, l, b)
        corr = jnp.einsum("dlb,dmb->dlm", x, x)  # (d, l, l)
        i = jnp.arange(SIZE)
        corr = corr.at[:, i, i].set(1.0)
        Ds, Qs = jnp.linalg.eigh(corr)  # Ds: (d, l), Qs: (d, l, l)
        Ds = jnp.sqrt(jnp.maximum(Ds, 0.0))
        Qs = Ds[:, None, :] * Qs
        return np.asarray(Qs)


def _build_nc():
    import concourse.bass as bass
    import concourse.tile as tile
    from concourse import bacc, mybir

    f32 = mybir.dt.float32
    f16 = mybir.dt.float16
    i8 = mybir.dt.int8
    WQ = GRP * SIZE    # qs columns per group (fp16)
    WN = GRP * BATCH   # noise/out columns per group (int8)
    nc = bacc.Bacc("TRN2", target_bir_lowering=False, debug=False,
                   num_devices=NCORES)
    qs = nc.dram_tensor("qs", [NGRP, SIZE, WQ], f16,
                        kind="ExternalInput").ap()
    nz = nc.dram_tensor("nz", [NGRP, SIZE, WN], i8,
                        kind="ExternalInput").ap()
    out = nc.dram_tensor("out", [NGRP, SIZE, WN], i8,
                         kind="ExternalOutput").ap()
    with tile.TileContext(nc) as tc:
        with (
            tc.tile_pool(name="qs", bufs=NGRP) as qsp,
            tc.tile_pool(name="nq", bufs=1) as nqp,
            tc.tile_pool(name="nf", bufs=NGRP) as nfp,
            tc.tile_pool(name="o", bufs=NGRP) as op_,
            tc.tile_pool(name="ps", bufs=4, space=bass.MemorySpace.PSUM) as pp,
        ):
            qts = [qsp.tile([SIZE, WQ], f16, name="qt")
                   for g in range(NGRP)]
            nfs = [nfp.tile([SIZE, WN], f16, name="nf")
                   for g in range(NGRP)]
            nq1 = nqp.tile([SIZE, WN], i8)
            # qs loads alternate the two HWDGE rings to prime faster.
            # noise: SWDGE cast-load int8(HBM) -> fp16(SBUF) for most
            # groups (costs fp16-side bytes on the SDMA pool, zero engine
            # time); group OFF ships plain int8 and DVE upconverts it (DVE
            # has slack), trimming the SDMA pool.  The last group loads in
            # chunks so its compute tail starts before the full group
            # lands.
            OFF = 1
            for g in range(NGRP):
                qeng = nc.sync if g % 2 == 0 else nc.scalar
                qeng.dma_start(qts[g], qs[g])
                if g == OFF:
                    nc.scalar.dma_start(nq1, nz[g])
            for g in range(NGRP - 1):
                if g != OFF:
                    nc.gpsimd.dma_start(nfs[g], nz[g])
            CH = WN // 4
            for c in range(4):
                nc.gpsimd.dma_start(nfs[NGRP - 1][:, c * CH:(c + 1) * CH],
                                    nz[NGRP - 1, :, c * CH:(c + 1) * CH])
            h = WN // 2
            for s in range(2):
                nc.vector.tensor_copy(nfs[OFF][:, s * h:(s + 1) * h],
                                      nq1[:, s * h:(s + 1) * h])
            for g in range(NGRP):
                qt, nf = qts[g], nfs[g]
                o = op_.tile([SIZE, WN], i8)
                # 4 matmuls share a [128, 1024] PSUM tile (2 banks); one
                # copy per quad, alternating ACT / DVE
                last = g == NGRP - 1
                for j in range(0, GRP, 4):
                    ps = pp.tile([SIZE, 4 * BATCH], f32)
                    for k in range(4):
                        wh = qt[:, (j + k) * SIZE:(j + k + 1) * SIZE]
                        xh = nf[:, (j + k) * BATCH:(j + k + 1) * BATCH]
                        nc.tensor.matmul(ps[:, k * BATCH:(k + 1) * BATCH],
                                         wh, xh, start=True, stop=True)
                    pcopy = (nc.scalar.copy if (j // 4) % 2 == 0
                             else nc.vector.tensor_copy)
                    pcopy(o[:, j * BATCH:(j + 4) * BATCH], ps[:])
                    if last:  # stream the last group out per quad
                        seng = nc.scalar if (j // 4) % 2 == 0 else nc.sync
                        seng.dma_start(out[g, :, j * BATCH:(j + 4) * BATCH],
                                       o[:, j * BATCH:(j + 4) * BATCH])
                if not last:
                    nc.sync.dma_start(out[g], o[:])
    nc.compile()
    return nc


def _run_device(qst: np.ndarray, noise_t: np.ndarray, trace: bool = False):
    """qst: (DIM, SIZE, SIZE) = QS transposed per dim (fp32);
    noise_t: (DIM, SIZE, BATCH) fp32.
    Returns (out_t (DIM, SIZE, BATCH) fp32, BassKernelResults)."""
    from concourse.bass_utils import run_bass_kernel_spmd

    if "nc" not in _cache:
        _cache["nc"] = _build_nc()
    nc = _cache["nc"]

    # qs: (DIM, SIZE, SIZE) -> per-core groups [NGRP, SIZE, GRP*SIZE] fp16
    q = qst.reshape(NCORES, NGRP, GRP, SIZE, SIZE).transpose(0, 1, 3, 2, 4)
    q = np.ascontiguousarray(q).reshape(NCORES, NGRP, SIZE, GRP * SIZE)
    q = q.astype(np.float16)
    # noise: quantize to int8 with scale 32, clip +-127
    nq = np.clip(np.rint(noise_t * QSCALE), -127, 127).astype(np.int8)
    n = nq.reshape(NCORES, NGRP, GRP, SIZE, BATCH).transpose(0, 1, 3, 2, 4)
    n = np.ascontiguousarray(n).reshape(NCORES, NGRP, SIZE, GRP * BATCH)
    in_maps = [{"qs": np.ascontiguousarray(q[c]),
                "nz": np.ascontiguousarray(n[c])} for c in range(NCORES)]
    res = run_bass_kernel_spmd(nc, in_maps, list(range(NCORES)), trace=trace)
    out_q = np.stack([res.results[c]["out"] for c in range(NCORES)])
    out_t = out_q.astype(np.float32) * (1.0 / QSCALE)
    out_t = out_t.reshape(NCORES, NGRP, SIZE, GRP, BATCH)
    out_t = out_t.transpose(0, 1, 3, 2, 4).reshape(DIM, SIZE, BATCH)
    return out_t, res


def kernel(standard_noise: np.ndarray, ref: np.ndarray) -> np.ndarray:
    qs = _host_qs(ref)  # (d, l, l)
    qst = np.ascontiguousarray(np.transpose(qs, (0, 2, 1)))
    noise_t = np.ascontiguousarray(
        np.transpose(np.asarray(standard_noise, dtype=np.float32), (2, 1, 0)))
    out_t, _ = _run_device(qst, noise_t)
    return np.ascontiguousarray(np.transpose(out_t, (2, 1, 0)))
